# revision 31
# baseline (speedup 1.0000x reference)
"""Trainium2 Bass kernel for the double-Alamouti ZF+SIC receiver (nn_MC_NCJT_RxUE).

Self-contained: builds the Bass program, shards batch across 8 NeuronCores,
runs via run_bass_kernel_spmd, reassembles full outputs.

Per (b,f) and symbol-pair p (notation: H = h[sym_a]+h[sym_b] = 2*h_avg,
r1 = ry[sym_a], r2 = ry[sym_b], sums over Nr=16 antennas):
  g0' = S |H0|^2+|H1|^2 ; g1' = S |H2|^2+|H3|^2       (gains_out = g'/4)
  alpha = S conj(H0)H2 + H1 conj(H3) ;  beta = S conj(H0)H3 - H1 conj(H2)
  u0 = S conj(H0)r1 + H1 conj(r2) ; u1 = S conj(H1)r1 - H0 conj(r2)
  v0 = S conj(H2)r1 + H3 conj(r2) ; v1 = S conj(H3)r1 - H2 conj(r2)
  d' = g0'g1' - |alpha|^2 - |beta|^2  (> 0)
  ZF: s_i = 2*N_i/d' :  N0 = g1'u0 - a v0 - b v1 ; N1 = g1'u1 + b* v0 - a* v1
                        N2 = g0'v0 - a* u0 + b u1 ; N3 = g0'v1 - b* u0 - a u1
  c0 = (g0' >= g1').  QAM16 bits: sign=(x<0), mag=(|x|>2/sqrt10); for s=2N/d':
  sign=(N<0), mag=(|N|*sqrt10 > d').  bx = (1-2bs)(1+2bm)/sqrt10 (better cluster).
  SIC (T-form): W' = sel(c0, (H2,H3), (H0,H1)) (=2*W), Cb = better per-symbol cols,
    T00 = S conj(W0')Cb0s1 + W1' conj(Cb1s2),  T01 = S conj(W0')Cb1s1 - W1' conj(Cb0s2)
    T10 = S conj(W1')Cb0s1 - W0' conj(Cb1s2),  T11 = S conj(W1')Cb1s1 + W0' conj(Cb0s2)
    M0 = sel(v0,u0) - bx0 T00 - bx1 T01 ; M1 = sel(v1,u1) - bx0 T10 - bx1 T11
    new_y = 2M/gw', gw' = sel(g1',g0') -> bits: (M<0), (|M|*sqrt10 > gw')
  bits0 = c0 ? bits(N0,N1) : bits(M) ;  bits1 = c0 ? bits(M) : bits(N2,N3)
"""
import sys
import os
import numpy as np

for _p in ("/opt/trn_rl_repo", "/root/.axon_site/_ro/trn_rl_repo"):
    if os.path.isdir(_p) and _p not in sys.path:
        sys.path.insert(0, _p)
        break

import concourse.bass as bass
import concourse.bacc as bacc
import concourse.mybir as mybir
from concourse import tile
from concourse.ap import AP

F32 = mybir.dt.float32
ALU = mybir.AluOpType
ACTF = mybir.ActivationFunctionType
AX = mybir.AxisListType

SQ10 = float(np.float32(np.sqrt(10.0)))
ISQ10 = float(np.float32(1.0 / np.sqrt(10.0)))

# (start_sym, n_pairs, sym_step, pair0, pair_step)
RUNS_A = ((0, 2, 12, 0, 5), (3, 4, 2, 1, 1))
RUNS_B = ((1, 2, 12, 0, 5), (4, 4, 2, 1, 1))


def _st(shape):
    s = [1] * len(shape)
    for i in range(len(shape) - 2, -1, -1):
        s[i] = s[i + 1] * shape[i + 1]
    return s


def _merge(dims):
    """Drop size-1 dims and merge adjacent dims preserving element order.

    Merge rule: outer (so, no) before inner (si, ni) combine when
    so == si*ni (contiguous run) or so == si == 0 (broadcast run).
    """
    out = []
    for (s, n) in dims:
        s, n = int(s), int(n)
        if n == 1:
            continue
        if out:
            so, no = out[-1]
            si, ni = s, n
            if so == si * ni or (so == 0 and si == 0):
                out[-1] = (si, no * ni)
                continue
        out.append((s, n))
    if not out:
        out = [(1, 1)]
    return out


class TW:
    """Tile wrapper: free-shape bookkeeping + explicit-AP views."""

    def __init__(self, pool, shape, tag, dtype=F32):
        self.shape = list(shape)          # free dims only (partition implied 128)
        self.strides = _st(self.shape)
        self.t = pool.tile([128] + self.shape, dtype, tag=tag)
        self.full = self.t[:]

    def v(self, off, *dims):
        """AP at element offset `off`, free dims auto-merged to minimal form."""
        return AP(self.full.tensor, self.full.offset + off,
                  [list(self.full.ap[0])] + [list(d) for d in _merge(dims)])

    def vr(self, off, *dims):
        """Raw view: dims kept exactly as given (for tensor_reduce semantics)."""
        return AP(self.full.tensor, self.full.offset + off,
                  [list(self.full.ap[0])] + [[int(s), int(n)] for (s, n) in dims])

    def off(self, *idx):
        """Element offset for a partial index tuple (leading dims)."""
        return sum(i * s for i, s in zip(idx, self.strides))

    def __getitem__(self, key):
        return self.t[key]


def build(BC, F, NB):
    nc = bacc.Bacc("TRN2", target_bir_lowering=False, debug=False)
    h_r = nc.dram_tensor("h_real", [BC, F, 14, 16, 4], F32, kind="ExternalInput")
    h_i = nc.dram_tensor("h_imag", [BC, F, 14, 16, 4], F32, kind="ExternalInput")
    ry_r = nc.dram_tensor("ry_real", [BC, F, 14, 16, 1], F32, kind="ExternalInput")
    ry_i = nc.dram_tensor("ry_imag", [BC, F, 14, 16, 1], F32, kind="ExternalInput")
    bits0 = nc.dram_tensor("bits0", [BC, F, 48], F32, kind="ExternalOutput")
    bits1 = nc.dram_tensor("bits1", [BC, F, 48], F32, kind="ExternalOutput")
    gains0 = nc.dram_tensor("gains0", [BC, F, 12], F32, kind="ExternalOutput")
    gains1 = nc.dram_tensor("gains1", [BC, F, 12], F32, kind="ExternalOutput")

    assert F % (128 * NB) == 0
    fgrp = F // (128 * NB)
    with tile.TileContext(nc) as tc:
        with (
            tc.tile_pool(name="inp", bufs=2) as ipool,
            tc.tile_pool(name="stg", bufs=2) as spool,
            tc.tile_pool(name="sml", bufs=2) as mpool,
        ):
            for grp in range(BC * fgrp):
                bi = grp // fgrp
                f0 = (grp % fgrp) * 128 * NB
                emit_group(nc, bi, f0, NB, F, h_r, h_i, ry_r, ry_i,
                           bits0, bits1, gains0, gains1, ipool, spool, mpool)
    return nc


def emit_group(nc, bi, f0, NB, F, h_r, h_i, ry_r, ry_i,
               bits0, bits1, gains0, gains1, ipool, spool, mpool):
    TT = nc.vector.tensor_tensor
    TS = nc.vector.tensor_scalar
    TC = nc.vector.tensor_copy
    RED = nc.vector.tensor_reduce
    PRED = nc.vector.copy_predicated
    SCP = nc.scalar.copy
    STT = nc.vector.scalar_tensor_tensor

    # ---------------- tiles ----------------
    ha = TW(ipool, (NB, 6, 2, 16, 4), "ha")      # (nb, pair, ri, r, t)
    hb = TW(ipool, (NB, 6, 2, 16, 4), "hb")
    H = TW(ipool, (NB, 6, 2, 16, 4), "H")
    rt = TW(ipool, (NB, 6, 4, 16), "rt")         # comps r1r,r1i,r2r,r2i
    rneg = TW(ipool, (NB, 6, 2, 16), "rneg")     # -r1r, -r2i

    stg_uv = TW(spool, (NB, 4, 6, 2, 16, 4), "stg_uv")   # (q, pair, half, r, t)
    stg_ab = TW(spool, (NB, 4, 6, 2, 16, 2), "stg_ab")   # (q, pair, half, r, s)
    stg_T = TW(spool, (NB, 4, 2, 6, 16, 2, 2), "stg_T")  # (q, sym, pair, r, w, cb)
    stg_g = TW(spool, (NB, 2, 6, 2, 16, 2), "stg_g")     # (tg, pair, ri, r, t)

    W = TW(ipool, (NB, 6, 2, 16, 2), "W")
    Cb1 = TW(ipool, (NB, 6, 2, 16, 2), "Cb1")
    Cb2 = TW(ipool, (NB, 6, 2, 16, 2), "Cb2")
    Ht23 = TW(ipool, (NB, 6, 2, 16, 2), "Ht23")
    hat01 = TW(ipool, (NB, 6, 2, 16, 2), "hat01")
    hbt01 = TW(ipool, (NB, 6, 2, 16, 2), "hbt01")
    U8 = mybir.dt.uint8
    mkA = TW(ipool, (NB, 6, 2, 16, 2), "mkA", dtype=U8)

    Ruv = TW(mpool, (NB, 4, 6, 4), "Ruv")        # (q, pair, t)
    Rabim = TW(mpool, (NB, 6, 2, 2), "Rabim")    # (pair, half, s)
    Rbre = TW(mpool, (NB, 6, 2), "Rbre")
    Rbim = TW(mpool, (NB, 6, 2), "Rbim")
    AB = TW(mpool, (NB, 6, 2, 2), "AB")          # (pair, a|b, re|im)
    Ut = TW(mpool, (NB, 6, 2, 2), "Ut")          # (pair, sym, ri)
    Vt = TW(mpool, (NB, 6, 2, 2), "Vt")
    g0t = TW(mpool, (NB, 6), "g0t")
    g1t = TW(mpool, (NB, 6), "g1t")
    c0 = TW(mpool, (NB, 6), "c0")
    mkB = TW(mpool, (NB, 6, 2, 2), "mkB", dtype=U8)
    mkC = TW(mpool, (NB, 6, 2, 4), "mkC", dtype=U8)
    mkD = TW(mpool, (NB, 6), "mkD", dtype=U8)
    Trer = TW(mpool, (NB, 2, 6, 2, 2), "Trer")   # (s, pair, w, cb)
    Tall = TW(mpool, (NB, 2, 4, 6, 2, 2), "Tall")  # (s, q, pair, w, cb)
    Dt = TW(mpool, (NB, 2, 6, 2, 2), "Dt")
    Tt = TW(mpool, (NB, 6, 2, 2, 2), "Tt")       # (pair, m, k, j)
    GNu = TW(mpool, (NB, 6, 2, 2), "GNu")        # (pair, s, j)
    GNv = TW(mpool, (NB, 6, 2, 2), "GNv")
    PN = TW(mpool, (NB, 4, 6, 2, 2, 2), "PN")    # (n, pair, k, i, j)
    Sre = TW(mpool, (NB, 4, 6, 2), "Sre")        # (n, pair, k)
    Dre = TW(mpool, (NB, 4, 6, 2), "Dre")
    Sim = TW(mpool, (NB, 4, 6, 2), "Sim")
    Dim = TW(mpool, (NB, 4, 6, 2), "Dim")
    Ntl = TW(mpool, (NB, 4, 6, 2), "Ntl")        # (n, pair, j)
    tmp1 = TW(mpool, (NB, 4, 6), "tmp1")
    tmp2 = TW(mpool, (NB, 4, 6), "tmp2")
    absq = TW(mpool, (NB, 6, 4), "absq")
    dpr = TW(mpool, (NB, 6), "dpr")
    Nab0 = TW(mpool, (NB, 6, 2, 2), "Nab0")      # |N|*sq10, cluster0 (pair, s, j)
    Nab1 = TW(mpool, (NB, 6, 2, 2), "Nab1")
    bs0 = TW(mpool, (NB, 6, 2, 2), "bs0")
    bs1 = TW(mpool, (NB, 6, 2, 2), "bs1")
    bm0 = TW(mpool, (NB, 6, 2, 2), "bm0")
    bm1 = TW(mpool, (NB, 6, 2, 2), "bm1")
    bselS = TW(mpool, (NB, 6, 2, 2), "bselS")
    bselM = TW(mpool, (NB, 6, 2, 2), "bselM")
    sgnv = TW(mpool, (NB, 6, 2, 2), "sgnv")
    bx = TW(mpool, (NB, 6, 2, 2), "bx")
    selVU = TW(mpool, (NB, 6, 2, 2), "selVU")
    PM = TW(mpool, (NB, 6, 2, 2, 2, 2), "PM")    # (pair, m, k, i, j)
    s1t = TW(mpool, (NB, 6, 2, 2), "s1t")        # (pair, m, k)
    s2t = TW(mpool, (NB, 6, 2, 2), "s2t")
    t1t = TW(mpool, (NB, 6, 2), "t1t")
    t2t = TW(mpool, (NB, 6, 2), "t2t")
    Mtl = TW(mpool, (NB, 6, 2, 2), "Mtl")        # (pair, m, j)
    Mabs = TW(mpool, (NB, 6, 2, 2), "Mabs")
    gw = TW(mpool, (NB, 6), "gw")
    msgn = TW(mpool, (NB, 6, 2, 2), "msgn")
    mmag = TW(mpool, (NB, 6, 2, 2), "mmag")
    cl0b = TW(mpool, (NB, 6, 2, 4), "cl0b")
    cl1b = TW(mpool, (NB, 6, 2, 4), "cl1b")
    newb = TW(mpool, (NB, 6, 2, 4), "newb")
    b0t = TW(mpool, (NB, 6, 2, 4), "b0t")
    b1t = TW(mpool, (NB, 6, 2, 4), "b1t")
    go0 = TW(mpool, (NB, 12), "go0")
    go1 = TW(mpool, (NB, 12), "go1")

    # ---------------- DMA in ----------------
    def dap(handle, off, *dims):
        fl = handle[:]
        return AP(fl.tensor, fl.offset + off,
                  [[int(s), int(n)] for (s, n) in dims])

    # h dram strides (elements): [F*896, 896, 64, 4, 1]; ry: [F*224, 224, 16, 1, 1]
    # input DMAs issue from the idle PE queue; ry/outs from SP queue.
    def dma_h(dst, ri, runs, src):
        for (s0, npair, sst, p0, pst) in runs:
            dv = dap(src, bi * F * 896 + f0 * 896 + s0 * 64,
                     (896, 128), (128 * 896, NB), (sst * 64, npair), (1, 64))
            ov = dst.v(p0 * 128 + ri * 64, (dst.strides[0], NB),
                       (128 * pst, npair), (1, 64))
            nc.tensor.dma_start(out=ov, in_=dv)

    dma_h(ha, 0, RUNS_A, h_r)
    dma_h(ha, 1, RUNS_A, h_i)
    dma_h(hb, 0, RUNS_B, h_r)
    dma_h(hb, 1, RUNS_B, h_i)
    for (slot, runs, src) in ((0, RUNS_A, ry_r), (1, RUNS_A, ry_i),
                              (2, RUNS_B, ry_r), (3, RUNS_B, ry_i)):
        for (s0, npair, sst, p0, pst) in runs:
            dv = dap(src, bi * F * 224 + f0 * 224 + s0 * 16,
                     (224, 128), (128 * 224, NB), (sst * 16, npair), (1, 16))
            ov = rt.v(p0 * 64 + slot * 16, (rt.strides[0], NB),
                      (64 * pst, npair), (1, 16))
            nc.sync.dma_start(out=ov, in_=dv)

    # ---------------- stage 1 ----------------
    TT(out=H.full, in0=ha.full, in1=hb.full, op=ALU.add)
    nc.scalar.mul(out=rneg.v(0, (rneg.strides[0], NB), (32, 6), (1, 16)),
                  in_=rt.v(0, (rt.strides[0], NB), (64, 6), (1, 16)), mul=-1.0)
    nc.scalar.mul(out=rneg.v(16, (rneg.strides[0], NB), (32, 6), (1, 16)),
                  in_=rt.v(48, (rt.strides[0], NB), (64, 6), (1, 16)), mul=-1.0)

    nbH, nbR, nbRn = H.strides[0], rt.strides[0], rneg.strides[0]

    def hview(ri):
        return H.v(ri * 64, (nbH, NB), (128, 6), (4, 16), (1, 4))

    def rview(comp):
        return rt.v(comp * 16, (nbR, NB), (64, 6), (1, 16), (0, 4))

    def rnview(comp):
        return rneg.v(comp * 16, (nbRn, NB), (32, 6), (1, 16), (0, 4))

    nbUV = stg_uv.strides[0]
    for (q, half, a, b) in (
        (0, 0, hview(0), rview(0)),    # A  = Hr*r1r
        (0, 1, hview(1), rview(1)),    # B  = Hi*r1i
        (1, 0, hview(0), rview(1)),    # C  = Hr*r1i
        (1, 1, hview(1), rnview(0)),   # Dn = Hi*(-r1r)
        (2, 0, hview(0), rview(2)),    # E  = Hr*r2r
        (2, 1, hview(1), rview(3)),    # F  = Hi*r2i
        (3, 0, hview(0), rnview(1)),   # Gn = Hr*(-r2i)
        (3, 1, hview(1), rview(2)),    # K  = Hi*r2r
    ):
        ov = stg_uv.v(q * 768 + half * 64, (nbUV, NB), (128, 6), (4, 16), (1, 4))
        TT(out=ov, in0=a, in1=b, op=ALU.mult)

    # uv reduces -> Ruv[q][pair][t] ; reduce merged (half,r)=32 innermost
    nbRuv = Ruv.strides[0]
    for q in range(4):
        iv = stg_uv.vr(q * 768, (nbUV, NB), (128, 6), (1, 4), (4, 32))
        ov = Ruv.vr(q * 24, (nbRuv, NB), (4, 6), (1, 4))
        RED(out=ov, in_=iv, axis=AX.X, op=ALU.add)

    # alpha/beta products; para t-pairing (0,1)x(2,3); anti (0,1)x(3,2)
    def hpart(ri, tbase, tstep=1):
        return H.v(ri * 64 + tbase, (nbH, NB), (128, 6), (4, 16), (tstep, 2))

    nbAB = stg_ab.strides[0]
    for (q, half, a, b) in (
        (0, 0, hpart(0, 0), hpart(0, 2)),       # P1 rr
        (0, 1, hpart(1, 0), hpart(1, 2)),       # P2 ii
        (1, 0, hpart(0, 0), hpart(1, 2)),       # P3 ri
        (1, 1, hpart(1, 0), hpart(0, 2)),       # P4 ir
        (2, 0, hpart(0, 0), hpart(0, 3, -1)),   # Q1 rr anti
        (2, 1, hpart(1, 0), hpart(1, 3, -1)),   # Q2 ii anti
        (3, 0, hpart(0, 0), hpart(1, 3, -1)),   # Q3 ri anti
        (3, 1, hpart(1, 0), hpart(0, 3, -1)),   # Q4 ir anti
    ):
        ov = stg_ab.v(q * 384 + half * 32, (nbAB, NB), (64, 6), (2, 16), (1, 2))
        TT(out=ov, in0=a, in1=b, op=ALU.mult)

    # alpha_re = sum q0 (all): [pair, s(2), (h,r)=32] XY
    nbABt = AB.strides[0]
    RED(out=AB.vr(0, (nbABt, NB), (4, 6)),
        in_=stg_ab.vr(0, (nbAB, NB), (64, 6), (1, 2), (2, 32)),
        axis=AX.XY, op=ALU.add)
    # alpha_im partials (q1, keep half&s): split per half, reduce r
    for h in range(2):
        RED(out=Rabim.vr(h * 2, (Rabim.strides[0], NB), (4, 6), (1, 2)),
            in_=stg_ab.vr(384 + h * 32, (nbAB, NB), (64, 6), (1, 2), (2, 16)),
            axis=AX.X, op=ALU.add)
    # beta_re partials (q2, keep s): reduce merged (h,r)=32
    RED(out=Rbre.vr(0, (Rbre.strides[0], NB), (2, 6), (1, 2)),
        in_=stg_ab.vr(2 * 384, (nbAB, NB), (64, 6), (1, 2), (2, 32)),
        axis=AX.X, op=ALU.add)
    # beta_im partials (q3, keep half): reduce merged (r,s)=32
    RED(out=Rbim.vr(0, (Rbim.strides[0], NB), (2, 6), (1, 2)),
        in_=stg_ab.vr(3 * 384, (nbAB, NB), (64, 6), (32, 2), (1, 32)),
        axis=AX.X, op=ALU.add)

    nbRi = Rabim.strides[0]

    def rab(h, s):
        return Rabim.v(h * 2 + s, (nbRi, NB), (4, 6))

    TT(out=tmp1.v(0, (tmp1.strides[0], NB), (1, 6)), in0=rab(0, 0), in1=rab(0, 1),
       op=ALU.subtract)
    TT(out=tmp1.v(6, (tmp1.strides[0], NB), (1, 6)), in0=rab(1, 0), in1=rab(1, 1),
       op=ALU.subtract)
    TT(out=AB.v(1, (nbABt, NB), (4, 6)),
       in0=tmp1.v(0, (tmp1.strides[0], NB), (1, 6)),
       in1=tmp1.v(6, (tmp1.strides[0], NB), (1, 6)), op=ALU.subtract)
    TT(out=AB.v(2, (nbABt, NB), (4, 6)),
       in0=Rbre.v(0, (Rbre.strides[0], NB), (2, 6)),
       in1=Rbre.v(1, (Rbre.strides[0], NB), (2, 6)), op=ALU.subtract)
    TT(out=AB.v(3, (nbABt, NB), (4, 6)),
       in0=Rbim.v(0, (Rbim.strides[0], NB), (2, 6)),
       in1=Rbim.v(1, (Rbim.strides[0], NB), (2, 6)), op=ALU.subtract)

    # gains: squares of H[tg] (on ACT) then reduce
    nbG = stg_g.strides[0]
    for tg in range(2):
        hv = H.v(tg * 2, (nbH, NB), (128, 6), (64, 2), (4, 16), (1, 2))
        ov = stg_g.v(tg * 384, (nbG, NB), (64, 6), (32, 2), (2, 16), (1, 2))
        nc.scalar.activation(out=ov, in_=hv, func=ACTF.Square)
    RED(out=g0t.full, in_=stg_g.vr(0, (nbG, NB), (64, 6), (1, 64)), axis=AX.X, op=ALU.add)
    RED(out=g1t.full, in_=stg_g.vr(384, (nbG, NB), (64, 6), (1, 64)), axis=AX.X, op=ALU.add)

    TT(out=c0.full, in0=g0t.full, in1=g1t.full, op=ALU.is_ge)

    # masks (full-tile replicas of c0)
    nbC = c0.strides[0]
    TS(out=mkA.full, in0=c0.v(0, (nbC, NB), (1, 6), (0, 2), (0, 16), (0, 2)),
       scalar1=1.0, scalar2=None, op0=ALU.mult)
    TS(out=mkB.full, in0=c0.v(0, (nbC, NB), (1, 6), (0, 2), (0, 2)),
       scalar1=1.0, scalar2=None, op0=ALU.mult)
    TS(out=mkC.full, in0=c0.v(0, (nbC, NB), (1, 6), (0, 2), (0, 4)),
       scalar1=1.0, scalar2=None, op0=ALU.mult)
    TS(out=mkD.full, in0=c0.full, scalar1=1.0, scalar2=None, op0=ALU.mult)

    # W / Cb selects (full tiles only)
    def ht_view(src, tbase):
        return src.v(tbase, (src.strides[0], NB), (128, 6), (64, 2), (4, 16), (1, 2))

    SCP(out=Ht23.full, in_=ht_view(H, 2))
    SCP(out=hat01.full, in_=ht_view(ha, 0))
    SCP(out=hbt01.full, in_=ht_view(hb, 0))
    SCP(out=W.full, in_=ht_view(H, 0))
    PRED(out=W.full, mask=mkA.full, data=Ht23.full)
    SCP(out=Cb1.full, in_=ht_view(ha, 2))
    PRED(out=Cb1.full, mask=mkA.full, data=hat01.full)
    SCP(out=Cb2.full, in_=ht_view(hb, 2))
    PRED(out=Cb2.full, mask=mkA.full, data=hbt01.full)

    # T products: q in (rr, ii, ri, ir) ; per sym tile Cb1/Cb2
    nbW, nbT = W.strides[0], stg_T.strides[0]

    def wv(ri):
        return W.v(ri * 32, (nbW, NB), (64, 6), (2, 16), (1, 2), (0, 2))

    def cbv(cbt, rj):
        return cbt.v(rj * 32, (cbt.strides[0], NB), (64, 6), (2, 16), (1, 2))

    for (q, ri, rj) in ((0, 0, 0), (1, 1, 1), (2, 0, 1), (3, 1, 0)):
        for s, cbt in ((0, Cb1), (1, Cb2)):
            for w in range(2):
                ov = stg_T.v(q * 768 + s * 384 + w * 2,
                             (nbT, NB), (64, 6), (4, 16), (1, 2))
                TT(out=ov,
                   in0=W.v(ri * 32 + w, (nbW, NB), (64, 6), (2, 16), (0, 2)),
                   in1=cbv(cbt, rj), op=ALU.mult)

    # T reduces: Tall[s, q, pair, (w,cb)=4] = sum_r ; Trer = q0+q1 ; Dt = q2-q3
    nbTa = Tall.strides[0]
    for s in range(2):
        for q in range(4):
            RED(out=Tall.vr(s * 96 + q * 24, (nbTa, NB), (4, 6), (1, 4)),
                in_=stg_T.vr(q * 768 + s * 384, (nbT, NB), (64, 6), (1, 4), (4, 16)),
                axis=AX.X, op=ALU.add)
    TT(out=Trer.full, in0=Tall.v(0, (nbTa, NB), (96, 2), (4, 6), (1, 4)),
       in1=Tall.v(24, (nbTa, NB), (96, 2), (4, 6), (1, 4)), op=ALU.add)
    TT(out=Dt.full, in0=Tall.v(48, (nbTa, NB), (96, 2), (4, 6), (1, 4)),
       in1=Tall.v(72, (nbTa, NB), (96, 2), (4, 6), (1, 4)), op=ALU.subtract)

    nbDt = Dt.strides[0]

    def tsl(tw, s, w, cb):
        return tw.v(s * 24 + w * 2 + cb, (tw.strides[0], NB), (4, 6))

    nbTt = Tt.strides[0]

    def tout(m, k, j):
        return Tt.v(m * 4 + k * 2 + j, (nbTt, NB), (8, 6))

    TT(out=tout(0, 0, 0), in0=tsl(Trer, 0, 0, 0), in1=tsl(Trer, 1, 1, 1), op=ALU.add)
    TT(out=tout(0, 1, 0), in0=tsl(Trer, 0, 0, 1), in1=tsl(Trer, 1, 1, 0), op=ALU.subtract)
    TT(out=tout(1, 0, 0), in0=tsl(Trer, 0, 1, 0), in1=tsl(Trer, 1, 0, 1), op=ALU.subtract)
    TT(out=tout(1, 1, 0), in0=tsl(Trer, 0, 1, 1), in1=tsl(Trer, 1, 0, 0), op=ALU.add)
    TT(out=tout(0, 0, 1), in0=tsl(Dt, 0, 0, 0), in1=tsl(Dt, 1, 1, 1), op=ALU.subtract)
    TT(out=tout(0, 1, 1), in0=tsl(Dt, 0, 0, 1), in1=tsl(Dt, 1, 1, 0), op=ALU.add)
    TT(out=tout(1, 0, 1), in0=tsl(Dt, 0, 1, 0), in1=tsl(Dt, 1, 0, 1), op=ALU.add)
    TT(out=tout(1, 1, 1), in0=tsl(Dt, 0, 1, 1), in1=tsl(Dt, 1, 0, 0), op=ALU.subtract)

    # ---------------- UV assembly ----------------
    def ruv1(q, t):
        return Ruv.v(q * 24 + t, (nbRuv, NB), (4, 6))

    nbU = Ut.strides[0]
    for (dst, toff) in ((Ut, 0), (Vt, 2)):
        TT(out=dst.v(0, (nbU, NB), (4, 6)), in0=ruv1(0, toff + 0), in1=ruv1(2, toff + 1), op=ALU.add)
        TT(out=dst.v(2, (nbU, NB), (4, 6)), in0=ruv1(0, toff + 1), in1=ruv1(2, toff + 0), op=ALU.subtract)
        TT(out=dst.v(1, (nbU, NB), (4, 6)), in0=ruv1(1, toff + 0), in1=ruv1(3, toff + 1), op=ALU.add)
        TT(out=dst.v(3, (nbU, NB), (4, 6)), in0=ruv1(1, toff + 1), in1=ruv1(3, toff + 0), op=ALU.subtract)

    # ---------------- N assembly ----------------
    # GNu[pair,s,j] = g1t * Ut ; GNv = g0t * Vt
    TT(out=GNu.full, in0=g1t.v(0, (nbC, NB), (1, 6), (0, 2), (0, 2)), in1=Ut.full, op=ALU.mult)
    TT(out=GNv.full, in0=g0t.v(0, (nbC, NB), (1, 6), (0, 2), (0, 2)), in1=Vt.full, op=ALU.mult)

    # PN[n,pair,k,i,j] = AB[k,i] * UVsrc(n)[k(sym), j]
    nbPN = PN.strides[0]
    for n, uvsrc in ((0, Vt), (1, Vt), (2, Ut), (3, Ut)):
        for k in range(2):
            # k-term pairing: even n -> (alpha, beta); odd n -> (beta, alpha)
            ab_idx = k if n % 2 == 0 else 1 - k
            TT(out=PN.v(n * 48 + k * 4, (nbPN, NB), (8, 6), (2, 2), (1, 2)),
               in0=AB.v(ab_idx * 2, (nbABt, NB), (4, 6), (1, 2), (0, 2)),
               in1=uvsrc.v(k * 2, (nbU, NB), (4, 6), (0, 2), (1, 2)),
               op=ALU.mult)

    def pnv(i, j):
        return PN.v(i * 2 + j, (nbPN, NB), (48, 4), (8, 6), (4, 2))

    TT(out=Sre.full, in0=pnv(0, 0), in1=pnv(1, 1), op=ALU.add)
    TT(out=Dre.full, in0=pnv(0, 0), in1=pnv(1, 1), op=ALU.subtract)
    TT(out=Sim.full, in0=pnv(0, 1), in1=pnv(1, 0), op=ALU.add)
    TT(out=Dim.full, in0=pnv(0, 1), in1=pnv(1, 0), op=ALU.subtract)

    nbS = Sre.strides[0]

    def sd(tw, n, k):
        return tw.v(n * 12 + k, (nbS, NB), (2, 6))

    def gnin(n, j):
        tw = GNu if n < 2 else GNv
        s = n & 1
        return tw.v(s * 2 + j, (nbU, NB), (4, 6))

    nbNt = Ntl.strides[0]

    def nout(n, j):
        return Ntl.v(n * 12 + j, (nbNt, NB), (2, 6))

    nbt1 = tmp1.strides[0]

    def tq(tw, qi):
        return tw.v(qi * 6, (nbt1, NB), (1, 6))

    # n0: re t=Dre[k0]+Dre[k1], N=GN-t ; im t=Sim[k0]+Sim[k1], N=GN-t
    TT(out=tq(tmp1, 0), in0=sd(Dre, 0, 0), in1=sd(Dre, 0, 1), op=ALU.add)
    TT(out=nout(0, 0), in0=gnin(0, 0), in1=tq(tmp1, 0), op=ALU.subtract)
    TT(out=tq(tmp2, 0), in0=sd(Sim, 0, 0), in1=sd(Sim, 0, 1), op=ALU.add)
    TT(out=nout(0, 1), in0=gnin(0, 1), in1=tq(tmp2, 0), op=ALU.subtract)
    # n1: re t=Sre[k0]-Sre[k1], N=GN+t ; im t=Dim[k0]-Dim[k1], N=GN+t
    TT(out=tq(tmp1, 1), in0=sd(Sre, 1, 0), in1=sd(Sre, 1, 1), op=ALU.subtract)
    TT(out=nout(1, 0), in0=gnin(1, 0), in1=tq(tmp1, 1), op=ALU.add)
    TT(out=tq(tmp2, 1), in0=sd(Dim, 1, 0), in1=sd(Dim, 1, 1), op=ALU.subtract)
    TT(out=nout(1, 1), in0=gnin(1, 1), in1=tq(tmp2, 1), op=ALU.add)
    # n2: re t=Dre[k1]-Sre[k0], N=GN+t ; im t=Sim[k1]-Dim[k0], N=GN+t
    TT(out=tq(tmp1, 2), in0=sd(Dre, 2, 1), in1=sd(Sre, 2, 0), op=ALU.subtract)
    TT(out=nout(2, 0), in0=gnin(2, 0), in1=tq(tmp1, 2), op=ALU.add)
    TT(out=tq(tmp2, 2), in0=sd(Sim, 2, 1), in1=sd(Dim, 2, 0), op=ALU.subtract)
    TT(out=nout(2, 1), in0=gnin(2, 1), in1=tq(tmp2, 2), op=ALU.add)
    # n3: re t=Sre[k0]+Dre[k1], N=GN-t ; im t=Dim[k0]+Sim[k1], N=GN-t
    TT(out=tq(tmp1, 3), in0=sd(Sre, 3, 0), in1=sd(Dre, 3, 1), op=ALU.add)
    TT(out=nout(3, 0), in0=gnin(3, 0), in1=tq(tmp1, 3), op=ALU.subtract)
    TT(out=tq(tmp2, 3), in0=sd(Dim, 3, 0), in1=sd(Sim, 3, 1), op=ALU.add)
    TT(out=nout(3, 1), in0=gnin(3, 1), in1=tq(tmp2, 3), op=ALU.subtract)

    # dprime
    TT(out=absq.full, in0=AB.full, in1=AB.full, op=ALU.mult)
    RED(out=tmp1.v(0, (nbt1, NB), (1, 6)), in_=absq.full, axis=AX.X, op=ALU.add)
    TT(out=tmp2.v(0, (nbt1, NB), (1, 6)), in0=g0t.full, in1=g1t.full, op=ALU.mult)
    TT(out=dpr.full, in0=tmp2.v(0, (nbt1, NB), (1, 6)), in1=tmp1.v(0, (nbt1, NB), (1, 6)),
       op=ALU.subtract)

    # N bits per cluster (cluster0 = N0,N1 ; cluster1 = N2,N3)
    # Ntl[n, pair, j]: view per cluster as (pair, s=n&1, j)
    def ncl(c):
        return Ntl.v(c * 24, (nbNt, NB), (2, 6), (12, 2), (1, 2))

    nc.scalar.activation(out=Nab0.full, in_=ncl(0), func=ACTF.Abs, scale=SQ10)
    nc.scalar.activation(out=Nab1.full, in_=ncl(1), func=ACTF.Abs, scale=SQ10)
    TS(out=bs0.full, in0=ncl(0), scalar1=0.0, scalar2=None, op0=ALU.is_lt)
    TS(out=bs1.full, in0=ncl(1), scalar1=0.0, scalar2=None, op0=ALU.is_lt)
    dprb = dpr.v(0, (nbC, NB), (1, 6), (0, 2), (0, 2))
    TT(out=bm0.full, in0=Nab0.full, in1=dprb, op=ALU.is_gt)
    TT(out=bm1.full, in0=Nab1.full, in1=dprb, op=ALU.is_gt)

    # better-cluster bits and bx
    SCP(out=bselS.full, in_=bs1.full)
    PRED(out=bselS.full, mask=mkB.full, data=bs0.full)
    SCP(out=bselM.full, in_=bm1.full)
    PRED(out=bselM.full, mask=mkB.full, data=bm0.full)
    TS(out=sgnv.full, in0=bselS.full, scalar1=-2.0, scalar2=1.0, op0=ALU.mult, op1=ALU.add)
    TS(out=bx.full, in0=bselM.full, scalar1=2.0, scalar2=1.0, op0=ALU.mult, op1=ALU.add)
    STT(out=bx.full, in0=sgnv.full, scalar=ISQ10, in1=bx.full, op0=ALU.mult, op1=ALU.mult)

    # selVU = c0 ? v : u
    SCP(out=selVU.full, in_=Ut.full)
    PRED(out=selVU.full, mask=mkB.full, data=Vt.full)

    # M: PM[pair,m,k,i,j] = bx[k,i]*Tt[m,k,j] ; split per (m,i) for <=3D APs
    nbPM, nbbx = PM.strides[0], bx.strides[0]
    for m in range(2):
        for i in range(2):
            TT(out=PM.v(m * 8 + i * 2, (nbPM, NB), (16, 6), (4, 2), (1, 2)),
               in0=bx.v(i, (nbbx, NB), (4, 6), (2, 2), (0, 2)),
               in1=Tt.v(m * 4, (nbTt, NB), (8, 6), (2, 2), (1, 2)),
               op=ALU.mult)
    TT(out=s1t.full, in0=PM.v(0, (nbPM, NB), (16, 6), (8, 2), (4, 2)),
       in1=PM.v(3, (nbPM, NB), (16, 6), (8, 2), (4, 2)), op=ALU.subtract)
    TT(out=s2t.full, in0=PM.v(1, (nbPM, NB), (16, 6), (8, 2), (4, 2)),
       in1=PM.v(2, (nbPM, NB), (16, 6), (8, 2), (4, 2)), op=ALU.add)
    nbs1 = s1t.strides[0]
    TT(out=t1t.full, in0=s1t.v(0, (nbs1, NB), (4, 6), (2, 2)),
       in1=s1t.v(1, (nbs1, NB), (4, 6), (2, 2)), op=ALU.add)
    TT(out=t2t.full, in0=s2t.v(0, (nbs1, NB), (4, 6), (2, 2)),
       in1=s2t.v(1, (nbs1, NB), (4, 6), (2, 2)), op=ALU.add)
    # M[pair,m,j]: j=0 from selVU[.,m,0]-t1 ; j=1 from selVU[.,m,1]-t2
    nbM = Mtl.strides[0]
    TT(out=Mtl.v(0, (nbM, NB), (4, 6), (2, 2)),
       in0=selVU.v(0, (nbU, NB), (4, 6), (2, 2)), in1=t1t.full, op=ALU.subtract)
    TT(out=Mtl.v(1, (nbM, NB), (4, 6), (2, 2)),
       in0=selVU.v(1, (nbU, NB), (4, 6), (2, 2)), in1=t2t.full, op=ALU.subtract)

    # gw = c0 ? g1t : g0t
    SCP(out=gw.full, in_=g0t.full)
    PRED(out=gw.full, mask=mkD.full, data=g1t.full)

    nc.scalar.activation(out=Mabs.full, in_=Mtl.full, func=ACTF.Abs, scale=SQ10)
    TS(out=msgn.full, in0=Mtl.full, scalar1=0.0, scalar2=None, op0=ALU.is_lt)
    TT(out=mmag.full, in0=Mabs.full, in1=gw.v(0, (nbC, NB), (1, 6), (0, 2), (0, 2)),
       op=ALU.is_gt)

    # bit tiles [pair, sym, (sign_re, sign_im, mag_re, mag_im)]
    nbB = cl0b.strides[0]
    for (dst, bsx, bmx) in ((cl0b, bs0, bm0), (cl1b, bs1, bm1), (newb, msgn, mmag)):
        TC(out=dst.v(0, (nbB, NB), (8, 6), (4, 2), (1, 2)), in_=bsx.full)
        TC(out=dst.v(2, (nbB, NB), (8, 6), (4, 2), (1, 2)), in_=bmx.full)

    SCP(out=b0t.full, in_=newb.full)
    PRED(out=b0t.full, mask=mkC.full, data=cl0b.full)
    SCP(out=b1t.full, in_=cl1b.full)
    PRED(out=b1t.full, mask=mkC.full, data=newb.full)

    # gains out (repeat over sym, scale 1/4)
    nc.scalar.activation(out=go0.full, in_=g0t.v(0, (nbC, NB), (1, 6), (0, 2)),
                         func=ACTF.Copy, scale=0.25)
    nc.scalar.activation(out=go1.full, in_=g1t.v(0, (nbC, NB), (1, 6), (0, 2)),
                         func=ACTF.Copy, scale=0.25)

    # ---------------- debug dumps ----------------
    if os.environ.get("KERNEL_DEBUG") and bi == 0 and f0 == 0:
        for (nm, tw) in (("Ntl", Ntl), ("AB", AB), ("Ut", Ut), ("Vt", Vt),
                         ("Tt", Tt), ("Mtl", Mtl), ("dpr", dpr), ("c0", c0),
                         ("bx", bx), ("selVU", selVU), ("gw", gw),
                         ("Ruv", Ruv), ("g0t", g0t), ("g1t", g1t),
                         ("Trer", Trer), ("Dt", Dt), ("W", W), ("Cb1", Cb1),
                         ("Cb2", Cb2), ("bs0", bs0), ("bm0", bm0)):
            tot = int(np.prod(tw.shape[1:]))
            dt = nc.dram_tensor(f"dbg_{nm}", [128, tot], F32, kind="ExternalOutput")
            nc.sync.dma_start(out=dt[:], in_=tw.v(0, (1, tot)))

    # ---------------- DMA out ----------------
    for (dst, src) in ((bits0, b0t), (bits1, b1t)):
        dv = dap(dst, bi * F * 48 + f0 * 48, (48, 128), (128 * 48, NB), (1, 48))
        nc.sync.dma_start(out=dv, in_=src.v(0, (src.strides[0], NB), (1, 48)))
    for (dst, src) in ((gains0, go0), (gains1, go1)):
        dv = dap(dst, bi * F * 12 + f0 * 12, (12, 128), (128 * 12, NB), (1, 12))
        nc.sync.dma_start(out=dv, in_=src.full)


# ---------------------------------------------------------------- runtime ----
_CACHE = {}


def _get_nc(BC, F, NB):
    key = (BC, F, NB)
    if key not in _CACHE:
        nc = build(BC, F, NB)
        nc.compile()
        _CACHE[key] = nc
    return _CACHE[key]


def kernel(ry_real, ry_imag, h_real, h_imag):
    from concourse.bass_utils import run_bass_kernel_spmd
    B, F = h_real.shape[0], h_real.shape[1]
    NCORES = 8
    BC = B // NCORES
    NB = int(os.environ.get("KERNEL_NB", "1"))
    nc = _get_nc(BC, F, NB)
    in_maps = []
    for c in range(NCORES):
        sl = slice(c * BC, (c + 1) * BC)
        in_maps.append({
            "h_real": np.ascontiguousarray(h_real[sl]),
            "h_imag": np.ascontiguousarray(h_imag[sl]),
            "ry_real": np.ascontiguousarray(ry_real[sl]),
            "ry_imag": np.ascontiguousarray(ry_imag[sl]),
        })
    res = run_bass_kernel_spmd(nc, in_maps, list(range(NCORES))).results
    bits0 = np.concatenate([res[c]["bits0"] for c in range(NCORES)], axis=0)
    bits1 = np.concatenate([res[c]["bits1"] for c in range(NCORES)], axis=0)
    g0 = np.concatenate([res[c]["gains0"] for c in range(NCORES)], axis=0)
    g1 = np.concatenate([res[c]["gains1"] for c in range(NCORES)], axis=0)
    return (bits0, bits1, g0, g1, np.float32(0.4))
